# revision 17
# baseline (speedup 1.0000x reference)
"""PRDC precision-loss kernel for Trainium2, 8-core SPMD.

loss = -mean_j max_i sigmoid((radii_i - d_rf[i,j]) / 0.1)
radii_i = 6th-smallest entry of row i of the real-real distance matrix
(index 5 ascending, index 0 is the ~0 self distance).

Sharding: real rows split 8 ways. Core c computes
  (a) rr block [1024, 8192] -> per-row 6th-NN radius for its rows,
  (b) rf block TRANSPOSED [8192 fake, 1024 real] -> per-fake-column max of
      (radii_i - d) over its real rows only (free-axis reduce).
Host combines the 8 partial column maxes with np.maximum, then applies
sigmoid/mean (monotone => commutes with max).

Selection trick: the 6th-smallest d^2 within a row equals the 6th-largest
qq where qq = 2*R.Rb^T - r2_j + C (monotone decreasing map of d^2,
row-constant terms dropped). qq is pooled to per-16-column maxes on the
fly (DVE), then 5 exclude-and-max passes pick the 6th-largest distinct
value from the 512 pooled maxes.
"""

import sys

import numpy as np
import ml_dtypes

sys.path.insert(0, "/opt/trn_rl_repo")

N = 8192
D = 1024
NCORES = 8
BLK = N // NCORES  # 1024 real rows per core
KSEL = 5  # number of exclude passes; radius = 6th largest
TEMP = 0.1
EPS = 1e-8
CSH = 5000.0  # shift keeping the selection domain strictly positive

_prog = None


def _build_program(phases: int = 4, p4_mode: str = "nottr"):
    from concourse import bacc, bass, tile, mybir

    f32 = mybir.dt.float32
    bf16 = mybir.dt.bfloat16
    Alu = mybir.AluOpType
    Act = mybir.ActivationFunctionType
    Ax = mybir.AxisListType

    nc = bacc.Bacc(None)

    rt_d = nc.declare_dram_parameter("rt", [16, 128, 8, 512], bf16, isOutput=False)
    ft_d = nc.declare_dram_parameter("ft", [64, 128, 8, 128], bf16, isOutput=False)
    rtb_d = nc.declare_dram_parameter("rtb", [128, 8, BLK], bf16, isOutput=False)
    r2b_d = nc.declare_dram_parameter("r2b", [128, N], f32, isOutput=False)
    r2bfr_d = nc.declare_dram_parameter("r2bfr", [128, BLK], f32, isOutput=False)
    r2part_d = nc.declare_dram_parameter("r2part", [128, 8], f32, isOutput=False)
    f2b_d = nc.declare_dram_parameter("f2b", [128, 64], f32, isOutput=False)
    ident_d = nc.declare_dram_parameter("ident", [128, 128], f32, isOutput=False)
    zmax_d = nc.declare_dram_parameter("zmax", [128, 64], f32, isOutput=True)
    radout_d = nc.declare_dram_parameter("radout", [1, BLK], f32, isOutput=True)

    with tile.TileContext(nc) as tc:
        with tc.tile_pool(name="persist", bufs=1) as pp:
            rtb_sb = pp.tile([128, 8, BLK], bf16, tag="rtb")
            r2b_sb = pp.tile([128, N], f32, tag="r2b")
            r2bfr_sb = pp.tile([128, BLK], f32, tag="r2bfr")
            r2part_sb = pp.tile([128, 8], f32, tag="r2part")
            f2b_sb = pp.tile([128, 64], f32, tag="f2b")
            ident_sb = pp.tile([128, 128], f32, tag="ident")
            ones_sb = pp.tile([1, 128], f32, tag="ones")
            m1_sb = pp.tile([128, 8, 512], f32, tag="m1")
            radii_sb = pp.tile([128, 8], f32, tag="radii")
            radrow_sb = pp.tile([1, BLK], f32, tag="radrow")
            radbf_sb = pp.tile([128, BLK], f32, tag="radbf")
            zmax_sb = pp.tile([128, 64], f32, tag="zmax")

            nc.sync.dma_start(rtb_sb[:], rtb_d[:])
            nc.sync.dma_start(r2b_sb[:], r2b_d[:])
            nc.sync.dma_start(r2bfr_sb[:], r2bfr_d[:])
            nc.sync.dma_start(r2part_sb[:], r2part_d[:])
            nc.sync.dma_start(f2b_sb[:], f2b_d[:])
            nc.sync.dma_start(ident_sb[:], ident_d[:])
            nc.gpsimd.memset(ones_sb[:], 1.0)

            if phases < 4:
                nc.gpsimd.memset(zmax_sb[:], 0.0)
            if phases < 3:
                nc.gpsimd.memset(radrow_sb[:], 0.0)

            # ---- Phase 1: real-real gram, pooled column maxes of qq ----
            if phases >= 1:
                _phase1(nc, tc, tile, mybir, rt_d, rtb_sb, r2b_sb, m1_sb)
            if phases >= 2:
                _phase2(nc, tc, tile, mybir, m1_sb, r2part_sb, radii_sb)
            if phases >= 3:
                _phase3(
                    nc, tc, tile, mybir, radii_sb, ident_sb, ones_sb,
                    radrow_sb, radbf_sb, radout_d,
                )
            if phases >= 4:
                _phase4(
                    nc, tc, tile, mybir, ft_d, rtb_sb, r2bfr_sb, f2b_sb,
                    radbf_sb, zmax_sb, p4_mode,
                )

            nc.sync.dma_start(zmax_d[:], zmax_sb[:])
            if phases < 3:
                nc.sync.dma_start(radout_d[:], radrow_sb[:])

    nc.finalize()
    return nc


def _phase1(nc, tc, tile, mybir, rt_d, rtb_sb, r2b_sb, m1_sb):
    f32 = mybir.dt.float32
    bf16 = mybir.dt.bfloat16
    Alu = mybir.AluOpType
    Ax = mybir.AxisListType
    if True:
        with (
                tc.tile_pool(name="rrmov", bufs=3) as movp,
                tc.tile_pool(name="rrps", bufs=8, space="PSUM") as rrps,
                tc.tile_pool(name="rrqq", bufs=4) as qqp,
            ):
                for cb in range(16):
                    mov = movp.tile([128, 8, 512], bf16)
                    nc.sync.dma_start(mov[:], rt_d[cb])
                    for r in range(8):
                        ps = rrps.tile([128, 512], f32)
                        for k in range(8):
                            nc.tensor.matmul(
                                ps[:],
                                rtb_sb[:, k, r * 128 : (r + 1) * 128],
                                mov[:, k, :],
                                start=(k == 0),
                                stop=(k == 7),
                            )
                        qq = qqp.tile([128, 32, 16], f32)
                        # qq = 2*g - (r2_j - C)
                        nc.vector.scalar_tensor_tensor(
                            qq[:],
                            ps[:],
                            2.0,
                            r2b_sb[:, cb * 512 : (cb + 1) * 512],
                            Alu.mult,
                            Alu.subtract,
                        )
                        nc.vector.reduce_max(
                            m1_sb[:, r, cb * 32 : (cb + 1) * 32], qq[:], axis=Ax.X
                        )


def _phase2(nc, tc, tile, mybir, m1_sb, r2part_sb, radii_sb):
    f32 = mybir.dt.float32
    Alu = mybir.AluOpType
    Act = mybir.ActivationFunctionType
    Ax = mybir.AxisListType
    # ---- Phase 2: 6th-largest via 5 exclude-and-max passes ----
    if True:
        with tc.tile_pool(name="selp", bufs=3) as selp:
                for r in range(8):
                    mcur = selp.tile([128, 1], f32, tag="m")
                    nc.vector.reduce_max(mcur[:], m1_sb[:, r, :], axis=Ax.X)
                    for _ in range(KSEL):
                        tmp = selp.tile([128, 512], f32, tag="tmp", bufs=2)
                        nc.vector.scalar_tensor_tensor(
                            tmp[:],
                            m1_sb[:, r, :],
                            mcur[:],
                            m1_sb[:, r, :],
                            Alu.is_lt,
                            Alu.mult,
                        )
                        mnext = selp.tile([128, 1], f32, tag="m")
                        nc.vector.reduce_max(mnext[:], tmp[:], axis=Ax.X)
                        mcur = mnext
                    # radii = sqrt(r2_i + C + eps - m5)
                    nc.scalar.activation(
                        radii_sb[:, r : r + 1],
                        mcur[:],
                        Act.Sqrt,
                        bias=r2part_sb[:, r : r + 1],
                        scale=-1.0,
                    )


def _phase3(
    nc, tc, tile, mybir, radii_sb, ident_sb, ones_sb, radrow_sb, radbf_sb, radout_d
):
    f32 = mybir.dt.float32
    Act = mybir.ActivationFunctionType
    # ---- Phase 3: radii columns -> one row (via identity), broadcast ----
    if True:
        with (
                tc.tile_pool(name="tpps", bufs=2, space="PSUM") as tpp,
                tc.tile_pool(name="bcps", bufs=2, space="PSUM") as bcp,
            ):
                for h in range(2):
                    pst = tpp.tile([1, 512], f32)
                    for r4 in range(4):
                        rr_ = h * 4 + r4
                        # out[0, p] = radii_sb[p, rr_]
                        nc.tensor.matmul(
                            pst[:, r4 * 128 : (r4 + 1) * 128],
                            radii_sb[:, rr_ : rr_ + 1],
                            ident_sb[:],
                            start=True,
                            stop=True,
                        )
                    nc.scalar.activation(
                        radrow_sb[:, h * 512 : (h + 1) * 512], pst[:], Act.Copy
                    )
                nc.sync.dma_start(radout_d[:], radrow_sb[:])
                for h in range(2):
                    psb = bcp.tile([128, 512], f32)
                    nc.tensor.matmul(
                        psb[:],
                        ones_sb[:],
                        radrow_sb[:, h * 512 : (h + 1) * 512],
                        start=True,
                        stop=True,
                    )
                    nc.scalar.activation(
                        radbf_sb[:, h * 512 : (h + 1) * 512], psb[:], Act.Copy
                    )


def _phase4(
    nc, tc, tile, mybir, ft_d, rtb_sb, r2bfr_sb, f2b_sb, radbf_sb, zmax_sb,
    mode: str = "full",
):
    f32 = mybir.dt.float32
    bf16 = mybir.dt.bfloat16
    Alu = mybir.AluOpType
    Act = mybir.ActivationFunctionType
    Ax = mybir.AxisListType
    njb = 4 if mode == "fewer" else 64
    # ---- Phase 4: fake-real (transposed), fused z and column max ----
    if mode == "fewer":
        nc.gpsimd.memset(zmax_sb[:], 0.0)
    if True:
        with (
                tc.tile_pool(name="ftp", bufs=3) as ftp,
                tc.tile_pool(name="frps", bufs=3, space="PSUM") as frps,
                tc.tile_pool(name="frsb", bufs=3) as frsb,
            ):
                for jb in range(njb):
                    ftt = ftp.tile([128, 8, 128], bf16)
                    dmaeng = nc.sync if mode == "syncdma" else nc.scalar
                    dmaeng.dma_start(ftt[:], ft_d[jb])
                    ps2 = frps.tile([128, 1024], f32)
                    for h in range(2):
                        for k in range(8):
                            nc.tensor.matmul(
                                ps2[:, h * 512 : (h + 1) * 512],
                                ftt[:, k, :],
                                rtb_sb[:, k, h * 512 : (h + 1) * 512],
                                start=(k == 0),
                                stop=(k == 7),
                            )
                    qf = frsb.tile([128, 1024], f32, tag="qf")
                    # qf = 2*g' - (r2_i - C)
                    nc.vector.scalar_tensor_tensor(
                        qf[:], ps2[:], 2.0, r2bfr_sb[:], Alu.mult, Alu.subtract
                    )
                    df = frsb.tile([128, 1024], f32, tag="df")
                    # d = sqrt(f2_j + C + eps - qf) = sqrt(r2_i + f2_j - 2g' + eps)
                    nc.scalar.activation(
                        df[:],
                        qf[:],
                        Act.Sqrt,
                        bias=f2b_sb[:, jb : jb + 1],
                        scale=-1.0,
                    )
                    zf = frsb.tile([128, 1024], f32, tag="zf")
                    if mode == "nottr":
                        nc.vector.tensor_tensor(
                            zf[:], radbf_sb[:], df[:], Alu.subtract
                        )
                        nc.vector.reduce_max(
                            zmax_sb[:, jb : jb + 1], zf[:], axis=Ax.X
                        )
                    else:
                        nc.vector.tensor_tensor_reduce(
                            zf[:],
                            radbf_sb[:],
                            df[:],
                            1.0,
                            -1e30,
                            Alu.subtract,
                            Alu.max,
                            zmax_sb[:, jb : jb + 1],
                        )


def _get_program():
    global _prog
    if _prog is None:
        _prog = _build_program()
    return _prog


def _prep_inputs(real_features: np.ndarray, fake_features: np.ndarray):
    bf16 = ml_dtypes.bfloat16
    R = np.ascontiguousarray(real_features, dtype=np.float32)
    F = np.ascontiguousarray(fake_features, dtype=np.float32)
    r2 = np.sum(R.astype(np.float64) ** 2, axis=1)
    f2 = np.sum(F.astype(np.float64) ** 2, axis=1)

    rt_prep = R.T.reshape(8, 128, N).transpose(1, 0, 2)  # [p, k, j]
    rt_in = rt_prep.reshape(128, 8, 16, 512).transpose(2, 0, 1, 3).astype(bf16)
    ft_prep = F.T.reshape(8, 128, N).transpose(1, 0, 2)
    ft_in = ft_prep.reshape(128, 8, 64, 128).transpose(2, 0, 1, 3).astype(bf16)

    r2b = np.broadcast_to((r2 - CSH).astype(np.float32), (128, N)).copy()
    f2b = (f2.reshape(64, 128).T + CSH + EPS).astype(np.float32)
    ident = np.eye(128, dtype=np.float32)

    in_maps = []
    for c in range(NCORES):
        Rb = R[c * BLK : (c + 1) * BLK]
        rtb_in = np.ascontiguousarray(
            Rb.T.reshape(8, 128, BLK).transpose(1, 0, 2)
        ).astype(bf16)
        r2blk = r2[c * BLK : (c + 1) * BLK]
        r2bfr = np.broadcast_to((r2blk - CSH).astype(np.float32), (128, BLK)).copy()
        r2part = np.ascontiguousarray(
            (r2blk.reshape(8, 128).T + CSH + EPS).astype(np.float32)
        )
        in_maps.append(
            {
                "rt": rt_in,
                "ft": ft_in,
                "rtb": rtb_in,
                "r2b": r2b,
                "r2bfr": r2bfr,
                "r2part": r2part,
                "f2b": f2b,
                "ident": ident,
            }
        )
    return in_maps


def _postprocess(results) -> np.ndarray:
    z = np.full(N, -np.inf, dtype=np.float64)
    for res in results:
        zc = np.asarray(res["zmax"], dtype=np.float64).T.reshape(-1)
        z = np.maximum(z, zc)
    loss = -np.mean(1.0 / (1.0 + np.exp(-z / TEMP)))
    return np.asarray(loss, dtype=np.float32)


def run_spmd(real_features: np.ndarray, fake_features: np.ndarray, **kwargs):
    """Run the SPMD kernel; returns (BassKernelResults, loss ndarray)."""
    from concourse.bass_utils import run_bass_kernel_spmd

    nc = _get_program()
    in_maps = _prep_inputs(real_features, fake_features)
    res = run_bass_kernel_spmd(nc, in_maps, list(range(NCORES)), **kwargs)
    return res, _postprocess(res.results)


def kernel(real_features: np.ndarray, fake_features: np.ndarray) -> np.ndarray:
    _, loss = run_spmd(real_features, fake_features)
    return loss


# revision 22
# speedup vs baseline: 1.2686x; 1.2686x over previous
"""PRDC precision-loss kernel for Trainium2, 8-core SPMD.

loss = -mean_j max_i sigmoid((radii_i - d_rf[i,j]) / 0.1)
radii_i = 6th-smallest entry of row i of the real-real distance matrix
(index 5 ascending, index 0 is the ~0 self distance).

Sharding: real rows split 8 ways. Core c computes
  (a) rr block [1024, 8192] -> per-row 6th-NN radius for its rows,
  (b) rf block TRANSPOSED [8192 fake, 1024 real] -> per-fake-column max of
      (radii_i - d) over its real rows only (free-axis reduce).
Host combines the 8 partial column maxes with np.maximum, then applies
sigmoid/mean (monotone => commutes with max).

Selection trick: the 6th-smallest d^2 within a row equals the 6th-largest
qq where qq = 2*R.Rb^T - r2_j + C (monotone decreasing map of d^2,
row-constant terms dropped). qq is pooled to per-16-column maxes on the
fly (DVE), then 5 exclude-and-max passes pick the 6th-largest distinct
value from the 512 pooled maxes.
"""

import sys

import numpy as np
import ml_dtypes

sys.path.insert(0, "/opt/trn_rl_repo")

N = 8192
D = 1024
NCORES = 8
BLK = N // NCORES  # 1024 real rows per core
KSEL = 5  # number of exclude passes; radius = 6th largest
TEMP = 0.1
EPS = 1e-8
CSH = 5000.0  # shift keeping the selection domain strictly positive

_prog = None


def _build_program(
    phases: int = 4, p4_mode: str = "nottr", p1_mode: str = "full", reps: int = 1
):
    from concourse import bacc, bass, tile, mybir

    f32 = mybir.dt.float32
    bf16 = mybir.dt.bfloat16
    Alu = mybir.AluOpType
    Act = mybir.ActivationFunctionType
    Ax = mybir.AxisListType

    nc = bacc.Bacc(None)

    rt_d = nc.declare_dram_parameter("rt", [16, 128, 8, 512], bf16, isOutput=False)
    ft_d = nc.declare_dram_parameter("ft", [64, 128, 8, 128], bf16, isOutput=False)
    rtb_d = nc.declare_dram_parameter("rtb", [128, 8, BLK], bf16, isOutput=False)
    r2b_d = nc.declare_dram_parameter("r2b", [128, N], f32, isOutput=False)
    r2bfr_d = nc.declare_dram_parameter("r2bfr", [128, BLK], f32, isOutput=False)
    r2part_d = nc.declare_dram_parameter("r2part", [128, 8], f32, isOutput=False)
    f2b_d = nc.declare_dram_parameter("f2b", [128, 64], f32, isOutput=False)
    ident_d = nc.declare_dram_parameter("ident", [128, 128], f32, isOutput=False)
    zmax_d = nc.declare_dram_parameter("zmax", [128, 64], f32, isOutput=True)
    radout_d = nc.declare_dram_parameter("radout", [1, BLK], f32, isOutput=True)

    with tile.TileContext(nc) as tc:
        with tc.tile_pool(name="persist", bufs=1) as pp:
            rtb_sb = pp.tile([128, 8, BLK], bf16, tag="rtb")
            r2b_sb = pp.tile([128, N], f32, tag="r2b")
            r2bfr_sb = pp.tile([128, BLK], f32, tag="r2bfr")
            r2part_sb = pp.tile([128, 8], f32, tag="r2part")
            f2b_sb = pp.tile([128, 64], f32, tag="f2b")
            ident_sb = pp.tile([128, 128], f32, tag="ident")
            ones_sb = pp.tile([1, 128], f32, tag="ones")
            m1_sb = pp.tile([128, 8, 512], f32, tag="m1")
            radii_sb = pp.tile([128, 8], f32, tag="radii")
            radrow_sb = pp.tile([1, BLK], f32, tag="radrow")
            radbf_sb = pp.tile([128, BLK], f32, tag="radbf")
            zmax_sb = pp.tile([128, 64], f32, tag="zmax")

            nc.sync.dma_start(rtb_sb[:], rtb_d[:])
            nc.sync.dma_start(r2b_sb[:], r2b_d[:])
            nc.sync.dma_start(r2bfr_sb[:], r2bfr_d[:])
            nc.sync.dma_start(r2part_sb[:], r2part_d[:])
            nc.sync.dma_start(f2b_sb[:], f2b_d[:])
            nc.sync.dma_start(ident_sb[:], ident_d[:])
            nc.gpsimd.memset(ones_sb[:], 1.0)

            if phases < 4:
                nc.gpsimd.memset(zmax_sb[:], 0.0)
            if phases < 3:
                nc.gpsimd.memset(radrow_sb[:], 0.0)

            for _rep in range(reps):
                if phases >= 1:
                    _phase1(
                        nc, tc, tile, mybir, rt_d, rtb_sb, r2b_sb, m1_sb, p1_mode
                    )
                if phases >= 2:
                    _phase2(nc, tc, tile, mybir, m1_sb, r2part_sb, radii_sb)
                if phases >= 3:
                    _phase3(
                        nc, tc, tile, mybir, radii_sb, ident_sb, ones_sb,
                        radrow_sb, radbf_sb, radout_d,
                    )
                if phases >= 4:
                    _phase4(
                        nc, tc, tile, mybir, ft_d, rtb_sb, r2bfr_sb, f2b_sb,
                        radbf_sb, zmax_sb, p4_mode,
                    )

            nc.sync.dma_start(zmax_d[:], zmax_sb[:])
            if phases < 3:
                nc.sync.dma_start(radout_d[:], radrow_sb[:])

    nc.finalize()
    return nc


def _phase1(nc, tc, tile, mybir, rt_d, rtb_sb, r2b_sb, m1_sb, mode: str = "full"):
    f32 = mybir.dt.float32
    bf16 = mybir.dt.bfloat16
    Alu = mybir.AluOpType
    Ax = mybir.AxisListType
    if True:
        with (
                tc.tile_pool(name="rrmov", bufs=3) as movp,
                tc.tile_pool(name="rrps", bufs=8, space="PSUM") as rrps,
                tc.tile_pool(name="rrqq", bufs=4) as qqp,
            ):
                for cb in range(16):
                    mov = movp.tile([128, 8, 512], bf16)
                    nc.sync.dma_start(mov[:], rt_d[cb])
                    if mode == "dmaonly":
                        continue
                    for r in range(8):
                        ps = rrps.tile([128, 512], f32)
                        for k in range(8):
                            nc.tensor.matmul(
                                ps[:],
                                rtb_sb[:, k, r * 128 : (r + 1) * 128],
                                mov[:, k, :],
                                start=(k == 0),
                                stop=(k == 7),
                            )
                        if mode == "nodve":
                            continue
                        qq = qqp.tile([128, 32, 16], f32)
                        # qq = 2*g - (r2_j - C)
                        nc.vector.scalar_tensor_tensor(
                            qq[:],
                            ps[:],
                            2.0,
                            r2b_sb[:, cb * 512 : (cb + 1) * 512],
                            Alu.mult,
                            Alu.subtract,
                        )
                        nc.vector.reduce_max(
                            m1_sb[:, r, cb * 32 : (cb + 1) * 32], qq[:], axis=Ax.X
                        )


def _phase2(nc, tc, tile, mybir, m1_sb, r2part_sb, radii_sb):
    f32 = mybir.dt.float32
    Alu = mybir.AluOpType
    Act = mybir.ActivationFunctionType
    Ax = mybir.AxisListType
    # ---- Phase 2: 6th-largest via 5 exclude-and-max passes ----
    if True:
        with tc.tile_pool(name="selp", bufs=3) as selp:
                for r in range(8):
                    mcur = selp.tile([128, 1], f32, tag="m")
                    nc.vector.reduce_max(mcur[:], m1_sb[:, r, :], axis=Ax.X)
                    for _ in range(KSEL):
                        tmp = selp.tile([128, 512], f32, tag="tmp", bufs=2)
                        nc.vector.scalar_tensor_tensor(
                            tmp[:],
                            m1_sb[:, r, :],
                            mcur[:],
                            m1_sb[:, r, :],
                            Alu.is_lt,
                            Alu.mult,
                        )
                        mnext = selp.tile([128, 1], f32, tag="m")
                        nc.vector.reduce_max(mnext[:], tmp[:], axis=Ax.X)
                        mcur = mnext
                    # radii = sqrt(r2_i + C + eps - m5)
                    nc.scalar.activation(
                        radii_sb[:, r : r + 1],
                        mcur[:],
                        Act.Sqrt,
                        bias=r2part_sb[:, r : r + 1],
                        scale=-1.0,
                    )


def _phase3(
    nc, tc, tile, mybir, radii_sb, ident_sb, ones_sb, radrow_sb, radbf_sb, radout_d
):
    f32 = mybir.dt.float32
    Act = mybir.ActivationFunctionType
    # ---- Phase 3: radii columns -> one row (via identity), broadcast ----
    if True:
        with (
                tc.tile_pool(name="tpps", bufs=2, space="PSUM") as tpp,
                tc.tile_pool(name="bcps", bufs=2, space="PSUM") as bcp,
            ):
                for h in range(2):
                    pst = tpp.tile([1, 512], f32)
                    for r4 in range(4):
                        rr_ = h * 4 + r4
                        # out[0, p] = radii_sb[p, rr_]
                        nc.tensor.matmul(
                            pst[:, r4 * 128 : (r4 + 1) * 128],
                            radii_sb[:, rr_ : rr_ + 1],
                            ident_sb[:],
                            start=True,
                            stop=True,
                        )
                    nc.scalar.activation(
                        radrow_sb[:, h * 512 : (h + 1) * 512], pst[:], Act.Copy
                    )
                nc.sync.dma_start(radout_d[:], radrow_sb[:])
                for h in range(2):
                    psb = bcp.tile([128, 512], f32)
                    nc.tensor.matmul(
                        psb[:],
                        ones_sb[:],
                        radrow_sb[:, h * 512 : (h + 1) * 512],
                        start=True,
                        stop=True,
                    )
                    nc.scalar.activation(
                        radbf_sb[:, h * 512 : (h + 1) * 512], psb[:], Act.Copy
                    )


def _phase4(
    nc, tc, tile, mybir, ft_d, rtb_sb, r2bfr_sb, f2b_sb, radbf_sb, zmax_sb,
    mode: str = "full",
):
    f32 = mybir.dt.float32
    bf16 = mybir.dt.bfloat16
    Alu = mybir.AluOpType
    Act = mybir.ActivationFunctionType
    Ax = mybir.AxisListType
    njb = 4 if mode == "fewer" else 64
    # ---- Phase 4: fake-real (transposed), fused z and column max ----
    if mode == "fewer":
        nc.gpsimd.memset(zmax_sb[:], 0.0)
    if True:
        with (
                tc.tile_pool(name="ftp", bufs=3) as ftp,
                tc.tile_pool(name="frps", bufs=3, space="PSUM") as frps,
                tc.tile_pool(name="frsb", bufs=3) as frsb,
            ):
                for jb in range(njb):
                    ftt = ftp.tile([128, 8, 128], bf16)
                    dmaeng = nc.sync if mode == "syncdma" else nc.scalar
                    dmaeng.dma_start(ftt[:], ft_d[jb])
                    ps2 = frps.tile([128, 1024], f32)
                    for h in range(2):
                        for k in range(8):
                            nc.tensor.matmul(
                                ps2[:, h * 512 : (h + 1) * 512],
                                ftt[:, k, :],
                                rtb_sb[:, k, h * 512 : (h + 1) * 512],
                                start=(k == 0),
                                stop=(k == 7),
                            )
                    qf = frsb.tile([128, 1024], f32, tag="qf")
                    # qf = 2*g' - (r2_i - C)
                    nc.vector.scalar_tensor_tensor(
                        qf[:], ps2[:], 2.0, r2bfr_sb[:], Alu.mult, Alu.subtract
                    )
                    df = frsb.tile([128, 1024], f32, tag="df")
                    # d = sqrt(f2_j + C + eps - qf) = sqrt(r2_i + f2_j - 2g' + eps)
                    nc.scalar.activation(
                        df[:],
                        qf[:],
                        Act.Sqrt,
                        bias=f2b_sb[:, jb : jb + 1],
                        scale=-1.0,
                    )
                    zf = frsb.tile([128, 1024], f32, tag="zf")
                    if mode == "nottr":
                        nc.vector.tensor_tensor(
                            zf[:], radbf_sb[:], df[:], Alu.subtract
                        )
                        nc.vector.reduce_max(
                            zmax_sb[:, jb : jb + 1], zf[:], axis=Ax.X
                        )
                    else:
                        nc.vector.tensor_tensor_reduce(
                            zf[:],
                            radbf_sb[:],
                            df[:],
                            1.0,
                            -1e30,
                            Alu.subtract,
                            Alu.max,
                            zmax_sb[:, jb : jb + 1],
                        )


def _get_program():
    global _prog
    if _prog is None:
        _prog = _build_program()
    return _prog


def _prep_inputs(real_features: np.ndarray, fake_features: np.ndarray):
    bf16 = ml_dtypes.bfloat16
    R = np.ascontiguousarray(real_features, dtype=np.float32)
    F = np.ascontiguousarray(fake_features, dtype=np.float32)
    r2 = np.sum(R.astype(np.float64) ** 2, axis=1)
    f2 = np.sum(F.astype(np.float64) ** 2, axis=1)

    rt_prep = R.T.reshape(8, 128, N).transpose(1, 0, 2)  # [p, k, j]
    rt_in = rt_prep.reshape(128, 8, 16, 512).transpose(2, 0, 1, 3).astype(bf16)
    ft_prep = F.T.reshape(8, 128, N).transpose(1, 0, 2)
    ft_in = ft_prep.reshape(128, 8, 64, 128).transpose(2, 0, 1, 3).astype(bf16)

    r2b = np.broadcast_to((r2 - CSH).astype(np.float32), (128, N)).copy()
    f2b = (f2.reshape(64, 128).T + CSH + EPS).astype(np.float32)
    ident = np.eye(128, dtype=np.float32)

    in_maps = []
    for c in range(NCORES):
        Rb = R[c * BLK : (c + 1) * BLK]
        rtb_in = np.ascontiguousarray(
            Rb.T.reshape(8, 128, BLK).transpose(1, 0, 2)
        ).astype(bf16)
        r2blk = r2[c * BLK : (c + 1) * BLK]
        r2bfr = np.broadcast_to((r2blk - CSH).astype(np.float32), (128, BLK)).copy()
        r2part = np.ascontiguousarray(
            (r2blk.reshape(8, 128).T + CSH + EPS).astype(np.float32)
        )
        in_maps.append(
            {
                "rt": rt_in,
                "ft": ft_in,
                "rtb": rtb_in,
                "r2b": r2b,
                "r2bfr": r2bfr,
                "r2part": r2part,
                "f2b": f2b,
                "ident": ident,
            }
        )
    return in_maps


def _postprocess(results) -> np.ndarray:
    z = np.full(N, -np.inf, dtype=np.float64)
    for res in results:
        zc = np.asarray(res["zmax"], dtype=np.float64).T.reshape(-1)
        z = np.maximum(z, zc)
    loss = -np.mean(1.0 / (1.0 + np.exp(-z / TEMP)))
    return np.asarray(loss, dtype=np.float32)


def run_spmd(real_features: np.ndarray, fake_features: np.ndarray, **kwargs):
    """Run the SPMD kernel; returns (BassKernelResults, loss ndarray)."""
    from concourse.bass_utils import run_bass_kernel_spmd

    nc = _get_program()
    in_maps = _prep_inputs(real_features, fake_features)
    res = run_bass_kernel_spmd(nc, in_maps, list(range(NCORES)), **kwargs)
    return res, _postprocess(res.results)


def kernel(real_features: np.ndarray, fake_features: np.ndarray) -> np.ndarray:
    _, loss = run_spmd(real_features, fake_features)
    return loss
